# revision 27
# baseline (speedup 1.0000x reference)
"""Bass/Trainium2 kernel for nn_Attn_81690277970335.

reference:  proj = enc @ W.T + b        [S, H]
            energies = proj @ hidden    [S]
            attn = softmax(energies)    [1, 1, S]

Key algebraic identity (exact in exact arithmetic):
            energies = enc @ (hidden @ W) + (b . hidden)
and softmax is invariant to the constant shift (b . hidden), so b drops out
entirely.  On device: v = hidden @ W (tiny), then energies = enc @ v.

Layout strategy: the host stages enc TRANSPOSED (h on partitions) in fp16, so
the big matvec runs on the otherwise-idle TensorEngine as 128x128-stationary
matmuls with N=1 (dot products down the partition axis), producing energies
directly in [128 seq, 32] layout for the softmax.  fp16 halves HBM traffic;
fp32 PSUM accumulation keeps rel-err ~3.5e-3 (gate is 2e-2).

Distribution: encoder_outputs sharded along seq across 8 cores; W, hidden
replicated (fp16).  Global softmax (exact two-pass) via one tiny ncfw
AllGather of per-core (max, sumexp) pairs, combined locally on every core.

The CC stream pays ~60us of lazy init per execution (CC-core wake ~20us +
~41us setup) before its first mesh can run, regardless of trigger time; a
dummy warm-up collective at kernel start absorbs that, so the stats
AllGather runs promptly once the dummy drains.  (A hand-rolled
remote_dma-based exchange would avoid ncfw entirely, but this container's
walrus build cannot encode the SWDGE ext-ISA instructions.)
"""

import sys

sys.path.insert(0, "/opt/trn_rl_repo")

import numpy as np

import concourse.bass as bass
import concourse.mybir as mybir
import concourse.tile as tile
from concourse.bass_utils import run_bass_kernel_spmd

SEQ = 32768
HID = 1024
NCORES = 8
SHARD = SEQ // NCORES  # 4096
P = 128  # partitions
KT = HID // P  # 8 h-chunks
MT = SHARD // P  # 32 seq tiles per shard
F32 = mybir.dt.float32
F16 = mybir.dt.float16
AL = mybir.AluOpType
ACT = mybir.ActivationFunctionType

_CACHE = {}


def _split_multiwaits(nc):
    """This container's walrus build accepts at most ONE sync-wait per
    instruction; Tile emits several.  Hoist extra waits onto single-wait
    NoOps inserted just before the instruction on the same engine queue
    (engines and DGE-issuing sequencers are in-order, so semantics hold)."""
    import bass_rust

    cnt = 0
    for f in nc.m.functions:
        for bb in f.blocks:
            il = bb.instructions
            i = 0
            while i < len(il):
                inst = il[i]
                si = inst.sync_info
                if si is not None and si.on_wait and len(si.on_wait) > 1:
                    waits = list(si.on_wait)
                    keep, extra = waits[-1], waits[:-1]
                    for j, w in enumerate(extra):
                        nop = mybir.InstNoOp(
                            name=f"{inst.name}-w{j}", ins=[], outs=[]
                        )
                        nop.engine = inst.engine
                        nop.sync_info = bass_rust.SyncInfo(
                            on_wait=[w], on_update=[]
                        )
                        il.insert(i, nop)
                        i += 1
                        cnt += 1
                    inst.sync_info = bass_rust.SyncInfo(
                        on_wait=[keep], on_update=list(si.on_update or [])
                    )
                i += 1
    return cnt


def _build_nc():
    nc = bass.Bass(num_devices=NCORES)

    # enc shard, transposed on host: encT[h, s] = enc[s, h], fp16
    encT = nc.dram_tensor("encT", [HID, SHARD], F16, kind="ExternalInput")
    # W row-blocks: wb[p, oc*HID + h] = W[oc*128 + p, h], fp16
    wb = nc.dram_tensor("wb", [P, KT * HID], F16, kind="ExternalInput")
    # hidden chunks: hb[p, oc] = hidden[oc*128 + p], fp16
    hb = nc.dram_tensor("hb", [P, KT], F16, kind="ExternalInput")
    # constants: [identity(128) | ones(128x128)] fp32
    cst = nc.dram_tensor("cst", [P, 2 * P], F32, kind="ExternalInput")
    out = nc.dram_tensor("attn", [P, MT], F32, kind="ExternalOutput")

    # collective bounce buffers (internal DRAM; output must be Shared)
    cc_in = nc.dram_tensor("cc_in", [1, 2], F32)
    cc_out = nc.dram_tensor("cc_out", [NCORES, 2], F32, addr_space="Shared")
    # dummy warm-up collective: singleton groups exercise the CC-stream
    # init without any cross-core wait, so the dummy drains ~7us earlier
    # than an 8-way warm-up would
    dummy_in = nc.dram_tensor("dummy_in", [1, 1], F32)
    dummy_out = nc.dram_tensor("dummy_out", [1, 1], F32, addr_space="Shared")

    with tile.TileContext(nc) as tc:
        with (
            tc.tile_pool(name="wpool", bufs=1) as wpool,
            tc.tile_pool(name="encp", bufs=1) as encp,
            tc.tile_pool(name="small", bufs=1) as small,
            tc.tile_pool(name="eps_p", bufs=1, space="PSUM") as eps_p,
            tc.tile_pool(name="vps_p", bufs=1, space="PSUM") as vps_p,
            tc.tile_pool(name="bps_p", bufs=1, space="PSUM") as bps_p,
        ):
            # ---- dummy collective to warm up the CC stream -----------------
            nc.gpsimd.collective_compute(
                "AllGather",
                AL.bypass,
                replica_groups=[[i] for i in range(NCORES)],
                ins=[dummy_in.ap().opt()],
                outs=[dummy_out.ap().opt()],
            )

            # ---- DMA queue setup: W halves lead both queues, enc behind ----
            dma_engs = [nc.sync, nc.scalar]
            wb_sb = wpool.tile([P, KT * HID], F16)
            hb_sb = wpool.tile([P, KT], F16)
            cst_sb = wpool.tile([P, 2 * P], F32)
            HW = KT * HID // 2
            nc.sync.dma_start(out=wb_sb[:, :HW], in_=wb[:, :HW])
            nc.scalar.dma_start(out=hb_sb[:], in_=hb[:])
            nc.scalar.dma_start(out=cst_sb[:], in_=cst[:])
            nc.scalar.dma_start(out=wb_sb[:, HW:], in_=wb[:, HW:])
            enc_sb = []
            for k in range(KT):
                t = encp.tile([P, SHARD], F16, name=f"enc{k}")
                dma_engs[k % 2].dma_start(
                    out=t[:], in_=encT[k * P : (k + 1) * P, :]
                )
                enc_sb.append(t)
            ident = cst_sb[:, 0:P]
            ones_row = cst_sb[0:1, P : 2 * P]  # [1, 128] of 1.0
            ones_col = cst_sb[:, P : P + 1]  # [128, 1] of 1.0

            # ---- v[k*128+j] = sum_o hidden[o] * W[o, k*128+j] ---------------
            # lhsT = W block [128 o, 128 j], rhs = hidden chunk [128 o, 1]
            # -> psum [128 j, k] accumulated over the 8 o-chunks.
            vps = vps_p.tile([P, KT], F32)
            for k in range(KT):
                for oc in range(KT):
                    # NB: start=True clears has_written bits for the WHOLE
                    # psum bank, so only the very first matmul of the bank
                    # may set it; start=False on a fresh element overwrites.
                    nc.tensor.matmul(
                        vps[:, k : k + 1],
                        wb_sb[:, oc * HID + k * P : oc * HID + (k + 1) * P],
                        hb_sb[:, oc : oc + 1],
                        start=(k == 0 and oc == 0),
                        stop=(k == KT - 1 and oc == KT - 1),
                        skip_group_check=True,
                    )
            vk_sb = small.tile([P, KT], F16)
            nc.vector.tensor_copy(vk_sb[:], vps[:])

            # ---- energies: eps[j, m] = enc_row(m*128+j) . v ----------------
            # lhsT = encT tile [128 h, 128 seq] (stationary), rhs = v chunk
            # [128 h, 1]; accumulate over the 8 h-chunks per column.
            eps = eps_p.tile([P, MT], F32)
            for k in range(KT):
                for m in range(MT):
                    nc.tensor.matmul(
                        eps[:, m : m + 1],
                        enc_sb[k][:, m * P : (m + 1) * P],
                        vk_sb[:, k : k + 1],
                        start=(k == 0 and m == 0),
                        stop=(k == KT - 1 and m == MT - 1),
                        skip_group_check=True,
                    )

            # ---- local softmax stats ---------------------------------------
            e_sb = small.tile([P, MT], F32)
            nc.vector.tensor_copy(e_sb[:], eps[:])
            lmax_p = small.tile([P, 1], F32)
            nc.vector.tensor_reduce(
                lmax_p[:], e_sb[:], axis=mybir.AxisListType.X, op=AL.max
            )
            # partition max: PE-transpose [128,1] -> [1,128], reduce on DVE
            # straight into row 0 of the remote-send tile (col 0).  Rows
            # 1-127 of cc_sb ride along in the sends as garbage; receivers
            # only read row 0.
            cc_sb = small.tile([P, 2], F32)
            tr_ps = bps_p.tile([1, P], F32)
            nc.tensor.transpose(tr_ps[:], lmax_p[:], ident)
            lmax = cc_sb[0:1, 0:1]
            nc.vector.tensor_reduce(
                lmax, tr_ps[:], axis=mybir.AxisListType.X, op=AL.max
            )
            # broadcast -lmax to all partitions: ones outer product on PE
            nm_ps = bps_p.tile([P, 1], F32)
            nc.tensor.matmul(nm_ps[:], ones_row, lmax, start=True, stop=True)
            nlmax_b = small.tile([P, 1], F32)
            nc.scalar.mul(nlmax_b[:], nm_ps[:], -1.0)

            # exp(E - lmax) and its per-partition sums in one ACT pass
            eexp = small.tile([P, MT], F32)
            lsum_p = small.tile([P, 1], F32)
            nc.scalar.activation(
                eexp[:], e_sb[:], ACT.Exp, bias=nlmax_b[:], accum_out=lsum_p[:]
            )
            # partition sum: ones matmul on PE, copied into cc col 1
            ls_ps = bps_p.tile([1, 1], F32)
            nc.tensor.matmul(ls_ps[:], lsum_p[:], ones_col, start=True, stop=True)
            lsum = cc_sb[0:1, 1:2]
            nc.vector.tensor_copy(lsum, ls_ps[:])

            # ---- exchange (lmax, lsum) across cores ------------------------
            # Single ncfw AllGather; deliberately the ONLY collective in the
            # kernel.  The CC stream pays its ~41us lazy-init from CC-core
            # boot (~20us in), concurrent with our main work, so the mesh can
            # begin as soon as both init and the doorbell are done.
            nc.gpsimd.dma_start(out=cc_in[:], in_=cc_sb[0:1, :])
            nc.gpsimd.collective_compute(
                "AllGather",
                AL.bypass,
                replica_groups=[list(range(NCORES))],
                ins=[cc_in.ap().opt()],
                outs=[cc_out.ap().opt()],
            )
            ag_sb = small.tile([1, 2 * NCORES], F32)
            nc.gpsimd.dma_start(out=ag_sb[:], in_=cc_out[:])
            ag3 = ag_sb[:].rearrange("p (r two) -> p r two", two=2)

            # gmax = max_r m_r
            gmax = small.tile([1, 1], F32)
            nc.vector.tensor_reduce(
                gmax[:], ag3[:, :, 0:1], axis=mybir.AxisListType.XY, op=AL.max
            )
            # gsum = sum_r s_r * exp(m_r - gmax)
            diffs = small.tile([1, NCORES], F32)
            nc.vector.tensor_scalar(
                out=diffs[:],
                in0=ag3[:, :, 0:1],
                scalar1=gmax[:],
                scalar2=None,
                op0=AL.subtract,
            )
            edifs = small.tile([1, NCORES], F32)
            nc.scalar.activation(edifs[:], diffs[:], ACT.Exp)
            prods = small.tile([1, NCORES], F32)
            nc.vector.tensor_tensor(
                out=prods[:], in0=edifs[:], in1=ag3[:, :, 1:2], op=AL.mult
            )
            gsum = small.tile([1, 1], F32)
            nc.vector.tensor_reduce(
                gsum[:], prods[:], axis=mybir.AxisListType.X, op=AL.add
            )

            # my rescale factor f = exp(lmax - gmax) / gsum
            d0 = small.tile([1, 1], F32)
            nc.vector.tensor_scalar(
                out=d0[:],
                in0=lmax,
                scalar1=gmax[:],
                scalar2=None,
                op0=AL.subtract,
            )
            e0 = small.tile([1, 1], F32)
            nc.scalar.activation(e0[:], d0[:], ACT.Exp)
            rg = small.tile([1, 1], F32)
            nc.vector.reciprocal(rg[:], gsum[:])
            f = small.tile([1, 1], F32)
            nc.vector.tensor_tensor(out=f[:], in0=e0[:], in1=rg[:], op=AL.mult)
            # broadcast f to all partitions on PE
            fb_ps = bps_p.tile([P, 1], F32)
            nc.tensor.matmul(fb_ps[:], ones_row, f[:], start=True, stop=True)
            f_b = small.tile([P, 1], F32)
            nc.scalar.copy(f_b[:], fb_ps[:])

            # ---- attn = eexp * f, store ------------------------------------
            attn_sb = small.tile([P, MT], F32)
            nc.scalar.mul(attn_sb[:], eexp[:], f_b[:])
            nc.sync.dma_start(out=out.ap(), in_=attn_sb[:])

    _split_multiwaits(nc)
    return nc


def _get_nc():
    if "nc" not in _CACHE:
        _CACHE["nc"] = _build_nc()
    return _CACHE["nc"]


def _prep_in_maps(hidden, encoder_outputs, W, b):
    hidden = np.asarray(hidden, dtype=np.float32)
    enc16 = np.asarray(encoder_outputs, dtype=np.float32).astype(np.float16)
    W16 = np.asarray(W, dtype=np.float32).astype(np.float16)
    # wb[p, oc*HID + h] = W[oc*128 + p, h]
    wb = np.ascontiguousarray(
        W16.reshape(KT, P, HID).transpose(1, 0, 2).reshape(P, KT * HID)
    )
    hb = np.ascontiguousarray(hidden.reshape(KT, P).T.astype(np.float16))
    cst = np.concatenate(
        [np.eye(P, dtype=np.float32), np.ones((P, P), dtype=np.float32)], axis=1
    )
    cst = np.ascontiguousarray(cst)
    in_maps = []
    for c in range(NCORES):
        encT = np.ascontiguousarray(enc16[c * SHARD : (c + 1) * SHARD].T)
        in_maps.append({"encT": encT, "wb": wb, "hb": hb, "cst": cst})
    return in_maps


def _ensure_ntff_hook():
    """Register the axon NTFF profile hook that this deployment's antenv
    package is missing, so trace=True yields a real HW profile."""
    import sys as _sys
    import types

    if "antenv.axon_hooks" in _sys.modules:
        return
    mod = types.ModuleType("antenv.axon_hooks")
    holder = [None]
    mod.set_axon_ntff_profile_hook = lambda h: holder.__setitem__(0, h)
    mod.get_axon_ntff_profile_hook = lambda: holder[0]
    _sys.modules["antenv.axon_hooks"] = mod
    import antenv

    antenv.axon_hooks = mod
    try:
        if "/root/.axon_site" not in _sys.path:
            _sys.path.insert(0, "/root/.axon_site")
        from trn_agent_boot.trn_boot import _ntff_profile_via_ctypes

        hook = _ntff_profile_via_ctypes("/opt/axon/libaxon_pjrt.so")
        if hook is not None:
            mod.set_axon_ntff_profile_hook(hook)
    except Exception as e:  # degrade to no tracing
        print(f"ntff hook registration failed: {e}", file=_sys.stderr)
    # artifact upload needs no external bucket for local profiling
    from concourse import bass_utils as _bu

    _bu.upload_artifacts = lambda tmpdir: tmpdir


def run(hidden, encoder_outputs, W, b, trace=False, **trace_kw):
    if trace:
        _ensure_ntff_hook()
    nc = _get_nc()
    in_maps = _prep_in_maps(hidden, encoder_outputs, W, b)
    res = run_bass_kernel_spmd(
        nc, in_maps, list(range(NCORES)), trace=trace, **trace_kw
    )
    # attn dram is [128, 32] with element [p, m] = attn(seq = m*128 + p)
    shards = [
        np.asarray(res.results[c]["attn"]).reshape(P, MT).T.reshape(-1)
        for c in range(NCORES)
    ]
    full = np.concatenate(shards).astype(np.float32)
    return full[None, None, :], res


def kernel(hidden, encoder_outputs, W, b):
    out, _ = run(hidden, encoder_outputs, W, b, trace=False)
    return out
